# revision 2
# baseline (speedup 1.0000x reference)
"""GridEmbedding kernel for Trainium2 (8 NeuronCores, vocab-sharded SPMD).

out[b,s,:] = emb_table[input_ids[b,s]]
           + grid_mask[b,s] * ((row_idx[b,s]+1)*row_vec + (col_idx[b,s]+1)*col_vec)

Sharding strategy: row-shard the vocab across the 8 cores (each core holds
a 6283-row slice of the table, cast to bf16) and redistribute tokens on
the host so every token is processed by the core that owns its embedding
row. The device does the real work — an indirect-DMA gather of one table
row per token plus the fused position-embedding add — and the host only
permutes index/metadata arrays and un-permutes the result. This cuts the
per-call device I/O footprint ~10x vs. replicating the f32 table
(445 MB/core -> ~45 MB/core), and bf16 halves the on-device HBM traffic.

Per core (C tokens padded to a multiple of 128, C/128 tiles):
  gpsimd: indirect-DMA gather of 128 bf16 embedding rows per tile (512KB)
  PE:     pos = coef[2,128]^T @ vecs[2,2048] into PSUM f32 (K=2 matmul)
  DVE:    tok(bf16) += pos(f32)
  sync:   HWDGE store of the bf16 tile; double-buffered via sem pipeline

Raw-bass implementation (explicit semaphores; this walrus build rejects
Tile's embedded multi-wait sync_info).
"""

import sys

for _p in ("/opt/trn_rl_repo",):
    if _p not in sys.path:
        sys.path.insert(0, _p)

import numpy as np
import ml_dtypes

B, S, H, VOCAB = 4, 8192, 2048, 50257
N_CORES = 8
TOK = B * S                  # 32768 tokens total
SLICE = (VOCAB + N_CORES - 1) // N_CORES   # 6283 vocab rows per core
P = 128                      # partitions / tokens per tile
MM_N = 512                   # matmul free-dim chunk (one PSUM bank)
NBUF = 6                     # token-tile double buffering depth
NPS = 2                      # PSUM buffers (4 banks each)

_PROGRAM_CACHE = {}
_EMB_CACHE = {}              # (shape, sig) -> per-core bf16 slices
LAST_RESULTS = None          # BassKernelResults of the most recent run


def build_program(ntiles, vocab_slice=SLICE, h=H, n_cores=N_CORES,
                  nbuf=None, num_swdge_queues=1):
    from concourse import bass, mybir

    tpc = ntiles * P
    nbuf = min(nbuf or NBUF, ntiles)
    nps = min(NPS, ntiles)
    nmm = h // MM_N

    nc = bass.Bass("TRN2", target_bir_lowering=False, debug=False,
                   num_devices=n_cores, num_swdge_queues=num_swdge_queues)

    emb = nc.dram_tensor("emb", [vocab_slice, h], mybir.dt.bfloat16,
                         kind="ExternalInput").ap()
    ids_d = nc.dram_tensor("idsT", [P, ntiles], mybir.dt.int32,
                           kind="ExternalInput").ap()
    rowcol = nc.dram_tensor("rowcol", [2, tpc], mybir.dt.int32,
                            kind="ExternalInput").ap()
    maskf = nc.dram_tensor("maskf", [2, tpc], mybir.dt.float32,
                           kind="ExternalInput").ap()
    vecs = nc.dram_tensor("vecs", [2, h], mybir.dt.float32,
                          kind="ExternalInput").ap()
    out = nc.dram_tensor("out", [tpc, h], mybir.dt.bfloat16,
                         kind="ExternalOutput").ap()

    from contextlib import ExitStack
    with ExitStack() as ctx:
        ids_sb_h = ctx.enter_context(
            nc.sbuf_tensor("ids_sb", [P, ntiles], mybir.dt.int32))
        rc_i_h = ctx.enter_context(
            nc.sbuf_tensor("rc_i", [2, tpc], mybir.dt.int32))
        mk_h = ctx.enter_context(
            nc.sbuf_tensor("mk", [2, tpc], mybir.dt.float32))
        vec_sb_h = ctx.enter_context(
            nc.sbuf_tensor("vec_sb", [2, h], mybir.dt.float32))
        coef_h = ctx.enter_context(
            nc.sbuf_tensor("coef", [2, tpc], mybir.dt.float32))
        tok_h = ctx.enter_context(
            nc.sbuf_tensor("tok", [P, nbuf * h], mybir.dt.bfloat16))
        pos_h = ctx.enter_context(
            nc.psum_tensor("pos", [P, nps * h], mybir.dt.float32))
        i_sem = ctx.enter_context(nc.semaphore("i_sem"))
        in_sem = ctx.enter_context(nc.semaphore("in_sem"))
        c_sem = ctx.enter_context(nc.semaphore("c_sem"))
        g_sems = [ctx.enter_context(nc.semaphore(f"g_sem{b}"))
                  for b in range(nbuf)]
        m_sems = [ctx.enter_context(nc.semaphore(f"m_sem{b}"))
                  for b in range(nps)]
        a_sem = ctx.enter_context(nc.semaphore("a_sem"))
        s_sems = [ctx.enter_context(nc.semaphore(f"s_sem{b}"))
                  for b in range(nbuf)]
        ids_sb = ids_sb_h.ap()
        rc_i = rc_i_h.ap()
        mk = mk_h.ap()
        vec_sb = vec_sb_h.ap()
        coef = coef_h.ap()
        tok = tok_h.ap()
        pos = pos_h.ap()

        def tokbuf(t):
            b = t % nbuf
            return tok[:, b * h:(b + 1) * h]

        def posbuf(t):
            b = t % nps
            return pos[:, b * h:(b + 1) * h]

        with nc.Block() as block:

            @block.sync
            def _(sync):
                # input loads (HWDGE FIFO: completion order = issue order)
                sync.dma_start(out=ids_sb, in_=ids_d).then_inc(i_sem, 16)
                sync.dma_start(out=rc_i, in_=rowcol).then_inc(in_sem, 16)
                sync.dma_start(out=mk, in_=maskf).then_inc(in_sem, 16)
                sync.dma_start(out=vec_sb, in_=vecs).then_inc(in_sem, 16)
                for t in range(ntiles):
                    sync.wait_ge(a_sem, t + 1)
                    sync.dma_start(out=out[P * t:P * (t + 1), :],
                                   in_=tokbuf(t)).then_inc(s_sems[t % nbuf], 16)
                for b in range(nbuf):
                    cnt = (ntiles - b + nbuf - 1) // nbuf
                    if cnt:
                        sync.wait_ge(s_sems[b], 16 * cnt)

            @block.gpsimd
            def _(gpsimd):
                gpsimd.wait_ge(i_sem, 16)  # ids_sb landed
                for t in range(ntiles):
                    if t >= nbuf:
                        gpsimd.wait_ge(s_sems[t % nbuf], 16 * (t // nbuf))
                    gpsimd.indirect_dma_start(
                        out=tokbuf(t), out_offset=None,
                        in_=emb,
                        in_offset=bass.IndirectOffsetOnAxis(
                            ap=ids_sb[:, t:t + 1], axis=0),
                    ).then_inc(g_sems[t % nbuf], 16)

            @block.vector
            def _(vector):
                vector.wait_ge(in_sem, 48)  # rc_i, mk, vecs landed
                # coef = (f32(rc_i) + 1) * mk, one fused DVE op
                vector.scalar_tensor_tensor(
                    out=coef, in0=rc_i, scalar=1.0, in1=mk,
                    op0=mybir.AluOpType.add,
                    op1=mybir.AluOpType.mult).then_inc(c_sem, 1)
                for t in range(ntiles):
                    vector.wait_ge(g_sems[t % nbuf], 16 * (t // nbuf + 1))
                    vector.wait_ge(m_sems[t % nps], nmm * (t // nps + 1))
                    vector.tensor_tensor(
                        out=tokbuf(t), in0=tokbuf(t), in1=posbuf(t),
                        op=mybir.AluOpType.add).then_inc(a_sem, 1)

            @block.tensor
            def _(tensor):
                tensor.wait_ge(c_sem, 1)
                for t in range(ntiles):
                    if t >= nps:
                        tensor.wait_ge(a_sem, t - nps + 1)
                    pb = posbuf(t)
                    for j in range(nmm):
                        tensor.matmul(
                            pb[:, MM_N * j:MM_N * (j + 1)],
                            coef[:, P * t:P * (t + 1)],
                            vec_sb[:, MM_N * j:MM_N * (j + 1)],
                        ).then_inc(m_sems[t % nps], 1)

    return nc


def _get_program(ntiles):
    if ntiles not in _PROGRAM_CACHE:
        _PROGRAM_CACHE[ntiles] = build_program(ntiles)
    return _PROGRAM_CACHE[ntiles]


def _emb_slices(emb_table):
    """Per-core bf16 vocab slices, padded to SLICE rows; cached."""
    emb = np.asarray(emb_table, dtype=np.float32)
    sig = (emb.shape, emb[0, :4].tobytes(), emb[-1, -4:].tobytes())
    if sig not in _EMB_CACHE:
        bf = emb.astype(ml_dtypes.bfloat16)
        padded = np.zeros((SLICE * N_CORES, H), dtype=ml_dtypes.bfloat16)
        padded[:VOCAB] = bf
        _EMB_CACHE.clear()
        _EMB_CACHE[sig] = [
            np.ascontiguousarray(padded[c * SLICE:(c + 1) * SLICE])
            for c in range(N_CORES)
        ]
    return _EMB_CACHE[sig]


def kernel(input_ids, row_idx, col_idx, grid_mask, emb_table, row_vec,
           col_vec):
    global LAST_RESULTS
    from concourse.bass_utils import run_bass_kernel_spmd

    ids = np.asarray(input_ids, dtype=np.int32).reshape(-1)
    row = np.asarray(row_idx, dtype=np.int32).reshape(-1)
    col = np.asarray(col_idx, dtype=np.int32).reshape(-1)
    mask = np.asarray(grid_mask).reshape(-1).astype(np.float32)

    # Redistribute tokens: core c owns vocab rows [c*SLICE, (c+1)*SLICE).
    owner = ids // SLICE
    order = np.argsort(owner, kind="stable")
    counts = np.bincount(owner, minlength=N_CORES)
    starts = np.zeros(N_CORES + 1, dtype=np.int64)
    np.cumsum(counts, out=starts[1:])
    cap = max(int(counts.max()), P)
    ntiles = -(-cap // P)
    C = ntiles * P

    ids_l = np.zeros((N_CORES, C), dtype=np.int32)
    rc = np.zeros((N_CORES, 2, C), dtype=np.int32)
    mk = np.zeros((N_CORES, 2, C), dtype=np.float32)
    sels = []
    for c in range(N_CORES):
        sel = order[starts[c]:starts[c + 1]]
        sels.append(sel)
        n = len(sel)
        ids_l[c, :n] = ids[sel] - c * SLICE
        rc[c, 0, :n] = row[sel]
        rc[c, 1, :n] = col[sel]
        mk[c, 0, :n] = mask[sel]
        mk[c, 1, :n] = mask[sel]

    vecs = np.concatenate([
        np.asarray(row_vec, dtype=np.float32).reshape(1, H),
        np.asarray(col_vec, dtype=np.float32).reshape(1, H),
    ], axis=0)
    slices = _emb_slices(emb_table)

    in_maps = []
    for c in range(N_CORES):
        ids_t = np.ascontiguousarray(ids_l[c].reshape(ntiles, P).T)
        in_maps.append({
            "emb": slices[c], "idsT": ids_t,
            "rowcol": np.ascontiguousarray(rc[c]),
            "maskf": np.ascontiguousarray(mk[c]),
            "vecs": vecs,
        })

    nc = _get_program(ntiles)
    res = run_bass_kernel_spmd(nc, in_maps, core_ids=list(range(N_CORES)))
    LAST_RESULTS = res

    out = np.empty((TOK, H), dtype=np.float32)
    for c in range(N_CORES):
        n = len(sels[c])
        out[sels[c]] = res.results[c]["out"][:n].astype(np.float32)
    return out.reshape(B, S, H)


# revision 5
# speedup vs baseline: 2.4465x; 2.4465x over previous
"""GridEmbedding kernel for Trainium2 (8 NeuronCores, vocab-sharded SPMD).

out[b,s,:] = emb_table[input_ids[b,s]]
           + grid_mask[b,s] * ((row_idx[b,s]+1)*row_vec + (col_idx[b,s]+1)*col_vec)

Sharding strategy: row-shard the vocab across the 8 cores (each core holds
a 6283-row slice of the table, cast to bf16) and redistribute tokens on
the host so every token is processed by the core that owns its embedding
row. The device does the real work — an indirect-DMA gather of one table
row per token plus the fused position-embedding add — and the host only
permutes index/metadata arrays and un-permutes the result. This cuts the
per-call device I/O footprint ~10x vs. replicating the f32 table
(445 MB/core -> ~45 MB/core), and bf16 halves the on-device HBM traffic.

Per core (C tokens padded to a multiple of 128, C/128 tiles):
  gpsimd: indirect-DMA gather of 128 bf16 embedding rows per tile (512KB)
  PE:     pos = coef[2,128]^T @ vecs[2,2048] into PSUM f32 (K=2 matmul)
  DVE:    tok(bf16) += pos(f32)
  sync:   HWDGE store of the bf16 tile; double-buffered via sem pipeline

Raw-bass implementation (explicit semaphores; this walrus build rejects
Tile's embedded multi-wait sync_info).
"""

import sys

for _p in ("/opt/trn_rl_repo",):
    if _p not in sys.path:
        sys.path.insert(0, _p)

import numpy as np
import ml_dtypes

B, S, H, VOCAB = 4, 8192, 2048, 50257
N_CORES = 8
TOK = B * S                  # 32768 tokens total
SLICE = (VOCAB + N_CORES - 1) // N_CORES   # 6283 vocab rows per core
P = 128                      # partitions / tokens per tile
MM_N = 512                   # matmul free-dim chunk (one PSUM bank)
NBUF = 6                     # token-tile double buffering depth
NPS = 2                      # PSUM buffers (4 banks each)

_PROGRAM_CACHE = {}
_EMB_CACHE = {}              # (shape, sig) -> per-core bf16 slices
LAST_RESULTS = None          # BassKernelResults of the most recent run


def build_program(ntiles, vocab_slice=SLICE, h=H, n_cores=N_CORES,
                  nbuf=None, num_swdge_queues=1):
    from concourse import bass, mybir

    tpc = ntiles * P
    nbuf = min(nbuf or NBUF, ntiles)
    nps = min(NPS, ntiles)
    nmm = h // MM_N

    nc = bass.Bass("TRN2", target_bir_lowering=False, debug=False,
                   num_devices=n_cores, num_swdge_queues=num_swdge_queues)

    emb = nc.dram_tensor("emb", [vocab_slice, h], mybir.dt.bfloat16,
                         kind="ExternalInput").ap()
    ids_d = nc.dram_tensor("idsT", [P, ntiles], mybir.dt.int32,
                           kind="ExternalInput").ap()
    rowcol = nc.dram_tensor("rowcol", [2, tpc], mybir.dt.int32,
                            kind="ExternalInput").ap()
    maskf = nc.dram_tensor("maskf", [2, tpc], mybir.dt.float32,
                           kind="ExternalInput").ap()
    vecs = nc.dram_tensor("vecs", [2, h], mybir.dt.bfloat16,
                          kind="ExternalInput").ap()
    out = nc.dram_tensor("out", [tpc, h], mybir.dt.bfloat16,
                         kind="ExternalOutput").ap()

    from contextlib import ExitStack
    with ExitStack() as ctx:
        ids_sb_h = ctx.enter_context(
            nc.sbuf_tensor("ids_sb", [P, ntiles], mybir.dt.int32))
        rc_i_h = ctx.enter_context(
            nc.sbuf_tensor("rc_i", [2, tpc], mybir.dt.int32))
        mk_h = ctx.enter_context(
            nc.sbuf_tensor("mk", [2, tpc], mybir.dt.float32))
        vec_sb_h = ctx.enter_context(
            nc.sbuf_tensor("vec_sb", [2, h], mybir.dt.bfloat16))
        coef_h = ctx.enter_context(
            nc.sbuf_tensor("coef", [2, tpc], mybir.dt.bfloat16))
        tok_h = ctx.enter_context(
            nc.sbuf_tensor("tok", [P, nbuf * h], mybir.dt.bfloat16))
        pos_h = ctx.enter_context(
            nc.psum_tensor("pos", [P, nps * h], mybir.dt.float32))
        i_sem = ctx.enter_context(nc.semaphore("i_sem"))
        in_sem = ctx.enter_context(nc.semaphore("in_sem"))
        c_sem = ctx.enter_context(nc.semaphore("c_sem"))
        g_sems = [ctx.enter_context(nc.semaphore(f"g_sem{b}"))
                  for b in range(nbuf)]
        m_sems = [ctx.enter_context(nc.semaphore(f"m_sem{b}"))
                  for b in range(nps)]
        a_sem = ctx.enter_context(nc.semaphore("a_sem"))
        s_sems = [ctx.enter_context(nc.semaphore(f"s_sem{b}"))
                  for b in range(nbuf)]
        ids_sb = ids_sb_h.ap()
        rc_i = rc_i_h.ap()
        mk = mk_h.ap()
        vec_sb = vec_sb_h.ap()
        coef = coef_h.ap()
        tok = tok_h.ap()
        pos = pos_h.ap()

        def tokbuf(t):
            b = t % nbuf
            return tok[:, b * h:(b + 1) * h]

        def posbuf(t):
            b = t % nps
            return pos[:, b * h:(b + 1) * h]

        with nc.Block() as block:

            @block.sync
            def _(sync):
                # input loads (HWDGE FIFO: completion order = issue order)
                sync.dma_start(out=ids_sb, in_=ids_d).then_inc(i_sem, 16)
                sync.dma_start(out=rc_i, in_=rowcol).then_inc(in_sem, 16)
                sync.dma_start(out=mk, in_=maskf).then_inc(in_sem, 16)
                sync.dma_start(out=vec_sb, in_=vecs).then_inc(in_sem, 16)
                for t in range(ntiles):
                    sync.wait_ge(a_sem, t + 1)
                    sync.dma_start(out=out[P * t:P * (t + 1), :],
                                   in_=tokbuf(t)).then_inc(s_sems[t % nbuf], 16)
                for b in range(nbuf):
                    cnt = (ntiles - b + nbuf - 1) // nbuf
                    if cnt:
                        sync.wait_ge(s_sems[b], 16 * cnt)

            @block.gpsimd
            def _(gpsimd):
                gpsimd.wait_ge(i_sem, 16)  # ids_sb landed
                for t in range(ntiles):
                    if t >= nbuf:
                        gpsimd.wait_ge(s_sems[t % nbuf], 16 * (t // nbuf))
                    gpsimd.indirect_dma_start(
                        out=tokbuf(t), out_offset=None,
                        in_=emb,
                        in_offset=bass.IndirectOffsetOnAxis(
                            ap=ids_sb[:, t:t + 1], axis=0),
                    ).then_inc(g_sems[t % nbuf], 16)

            @block.vector
            def _(vector):
                vector.wait_ge(in_sem, 48)  # rc_i, mk, vecs landed
                # coef = (f32(rc_i) + 1) * mk, one fused DVE op
                vector.scalar_tensor_tensor(
                    out=coef, in0=rc_i, scalar=1.0, in1=mk,
                    op0=mybir.AluOpType.add,
                    op1=mybir.AluOpType.mult).then_inc(c_sem, 1)
                for t in range(ntiles):
                    vector.wait_ge(g_sems[t % nbuf], 16 * (t // nbuf + 1))
                    vector.wait_ge(m_sems[t % nps], nmm * (t // nps + 1))
                    vector.tensor_tensor(
                        out=tokbuf(t), in0=tokbuf(t), in1=posbuf(t),
                        op=mybir.AluOpType.add).then_inc(a_sem, 1)

            @block.tensor
            def _(tensor):
                tensor.wait_ge(c_sem, 1)
                for t in range(ntiles):
                    if t >= nps:
                        tensor.wait_ge(a_sem, t - nps + 1)
                    pb = posbuf(t)
                    for j in range(nmm):
                        tensor.matmul(
                            pb[:, MM_N * j:MM_N * (j + 1)],
                            coef[:, P * t:P * (t + 1)],
                            vec_sb[:, MM_N * j:MM_N * (j + 1)],
                        ).then_inc(m_sems[t % nps], 1)

    return nc


def _get_program(ntiles):
    if ntiles not in _PROGRAM_CACHE:
        _PROGRAM_CACHE[ntiles] = build_program(ntiles)
    return _PROGRAM_CACHE[ntiles]


def _emb_slices(emb_table):
    """Per-core bf16 vocab slices, padded to SLICE rows; cached."""
    emb = np.asarray(emb_table, dtype=np.float32)
    sig = (emb.shape, emb[0, :4].tobytes(), emb[-1, -4:].tobytes())
    if sig not in _EMB_CACHE:
        bf = emb.astype(ml_dtypes.bfloat16)
        padded = np.zeros((SLICE * N_CORES, H), dtype=ml_dtypes.bfloat16)
        padded[:VOCAB] = bf
        _EMB_CACHE.clear()
        _EMB_CACHE[sig] = [
            np.ascontiguousarray(padded[c * SLICE:(c + 1) * SLICE])
            for c in range(N_CORES)
        ]
    return _EMB_CACHE[sig]


def kernel(input_ids, row_idx, col_idx, grid_mask, emb_table, row_vec,
           col_vec):
    global LAST_RESULTS
    from concourse.bass_utils import run_bass_kernel_spmd

    ids = np.asarray(input_ids, dtype=np.int32).reshape(-1)
    row = np.asarray(row_idx, dtype=np.int32).reshape(-1)
    col = np.asarray(col_idx, dtype=np.int32).reshape(-1)
    mask = np.asarray(grid_mask).reshape(-1).astype(np.float32)

    # Redistribute tokens: core c owns vocab rows [c*SLICE, (c+1)*SLICE).
    owner = ids // SLICE
    order = np.argsort(owner, kind="stable")
    counts = np.bincount(owner, minlength=N_CORES)
    starts = np.zeros(N_CORES + 1, dtype=np.int64)
    np.cumsum(counts, out=starts[1:])
    cap = max(int(counts.max()), P)
    ntiles = -(-cap // P)
    C = ntiles * P

    ids_l = np.zeros((N_CORES, C), dtype=np.int32)
    rc = np.zeros((N_CORES, 2, C), dtype=np.int32)
    mk = np.zeros((N_CORES, 2, C), dtype=np.float32)
    sels = []
    for c in range(N_CORES):
        sel = order[starts[c]:starts[c + 1]]
        sels.append(sel)
        n = len(sel)
        ids_l[c, :n] = ids[sel] - c * SLICE
        rc[c, 0, :n] = row[sel]
        rc[c, 1, :n] = col[sel]
        mk[c, 0, :n] = mask[sel]
        mk[c, 1, :n] = mask[sel]

    vecs = np.concatenate([
        np.asarray(row_vec, dtype=np.float32).reshape(1, H),
        np.asarray(col_vec, dtype=np.float32).reshape(1, H),
    ], axis=0).astype(ml_dtypes.bfloat16)
    slices = _emb_slices(emb_table)

    in_maps = []
    for c in range(N_CORES):
        ids_t = np.ascontiguousarray(ids_l[c].reshape(ntiles, P).T)
        in_maps.append({
            "emb": slices[c], "idsT": ids_t,
            "rowcol": np.ascontiguousarray(rc[c]),
            "maskf": np.ascontiguousarray(mk[c]),
            "vecs": vecs,
        })

    nc = _get_program(ntiles)
    res = run_bass_kernel_spmd(nc, in_maps, core_ids=list(range(N_CORES)))
    LAST_RESULTS = res

    out = np.empty((TOK, H), dtype=np.float32)
    for c in range(N_CORES):
        n = len(sels[c])
        out[sels[c]] = res.results[c]["out"][:n].astype(np.float32)
    return out.reshape(B, S, H)
